# revision 1
# baseline (speedup 1.0000x reference)
"""Trainium2 Bass kernel for nn_Grid_fun: out = tile(feat(z), 6) @ a.

Math: z = [x, 1] (N,4); feat = (z⊗z).reshape(N,16); out = tile(feat,6) @ a
    = feat @ a_eff  where a_eff = a.reshape(6,16,3).sum(0)   [16,3]
    => out[n,c] = z[n]^T A_c z[n],  A_c = a_eff[:,c].reshape(4,4)

Device algorithm (per core, data-parallel over N):
  Host stages x as Z[3g+j, m] = x[12 m + g, j]  (12 groups x 3 comps = 36
  partition rows, points along the free dim).
  mm1:  V[108,F] = P_V^T @ Z       (9 linear forms per group)
  ACT:  R = Square(V + bias)       (basis {X^2,Y^2,Z^2,(X+Y)^2,(X+Z)^2,
                                    (Y+Z)^2,(u_c.x+1)^2 c=0..2})
  mm2:  O[36,F] = A_blk^T @ R      (block-diag 9->3 per group), written at
        PSUM base partition 0 / 64 for alternating column tiles
  DVE:  out_sb = O + k_vec         (folds the constant term), DMA out.
The square basis exactly reproduces the quadratic + linear + constant parts:
  quad: 6 canonical squares; linear: w_c*(u_c.x+1)^2 with u_c = L_c/(2 k_c)
  (quadratic pollution subtracted via the canonical basis); const: k_c folded
  into the output copy.
"""

import sys

if "/opt/trn_rl_repo" not in sys.path:
    sys.path.insert(0, "/opt/trn_rl_repo")

from contextlib import ExitStack

import numpy as np

import concourse.bass as bass
import concourse.mybir as mybir
import concourse.tile as tile
from concourse import bacc
from concourse.bass_utils import run_bass_kernel_spmd

N_CORES = 8
N_POINTS = 1_000_000
N_PER_CORE = N_POINTS // N_CORES  # 125000
G = 12  # points (groups) per column
FTOT = 10418  # columns per core; G*FTOT = 125016 >= N_PER_CORE
NPAD = G * FTOT
FT = 512  # matmul free-dim tile
NTILES = (FTOT + 2 * FT - 1) // (2 * FT)  # macro tiles of 2*FT columns

_CACHE: dict = {}


def _build_nc():
    nc = bacc.Bacc("TRN2", target_bir_lowering=False)
    f32 = mybir.dt.float32

    z_d = nc.dram_tensor("z", [36, FTOT], f32, kind="ExternalInput")
    pv_d = nc.dram_tensor("pv", [36, 108], f32, kind="ExternalInput")
    ab_d = nc.dram_tensor("ab", [108, 36], f32, kind="ExternalInput")
    bias_d = nc.dram_tensor("bias", [108, 1], f32, kind="ExternalInput")
    kv_d = nc.dram_tensor("kv", [128, 1], f32, kind="ExternalInput")
    o_d = nc.dram_tensor("o", [72, FTOT // 2], f32, kind="ExternalOutput")

    with tile.TileContext(nc) as tc:
        with ExitStack() as ctx:
            cpool = ctx.enter_context(tc.tile_pool(name="consts", bufs=1))
            zpool = ctx.enter_context(tc.tile_pool(name="zt", bufs=3))
            rpool = ctx.enter_context(tc.tile_pool(name="rt", bufs=2))
            opool = ctx.enter_context(tc.tile_pool(name="ot", bufs=3))
            vpool = ctx.enter_context(
                tc.tile_pool(name="vps", bufs=2, space="PSUM")
            )
            ops_pool = ctx.enter_context(
                tc.tile_pool(name="ops", bufs=2, space="PSUM")
            )

            pv = cpool.tile([36, 108], f32)
            nc.gpsimd.dma_start(pv[:], pv_d[:, :])
            ab = cpool.tile([108, 36], f32)
            nc.gpsimd.dma_start(ab[:], ab_d[:, :])
            bias = cpool.tile([108, 1], f32)
            nc.gpsimd.dma_start(bias[:], bias_d[:, :])
            kv = cpool.tile([128, 1], f32)
            nc.gpsimd.dma_start(kv[:], kv_d[:, :])

            for t in range(NTILES):
                c0 = 2 * FT * t
                w = min(2 * FT, FTOT - c0)  # macro width (2*FT or tail)
                h = w // 2
                zt = zpool.tile([36, 2 * FT], f32)
                nc.sync.dma_start(zt[:, :w], z_d[:, c0 : c0 + w])

                vps = vpool.tile([108, 2 * FT], f32)
                nc.tensor.matmul(
                    vps[:, :h], pv[:], zt[:, :h], start=True, stop=True
                )
                nc.tensor.matmul(
                    vps[:, h:w], pv[:], zt[:, h:w], start=True, stop=True
                )

                rt = rpool.tile([108, 2 * FT], f32)
                nc.scalar.activation(
                    rt[:, :w],
                    vps[:, :w],
                    mybir.ActivationFunctionType.Square,
                    bias=bias[:],
                    scale=1.0,
                )

                ops = ops_pool.tile([128, FT], f32)
                nc.tensor.matmul(
                    ops[0:36, :h], ab[:], rt[:, :h], start=True, stop=True
                )
                nc.tensor.matmul(
                    ops[64:100, :h], ab[:], rt[:, h:w], start=True, stop=True
                )

                ot = opool.tile([128, FT], f32)
                nc.vector.tensor_scalar(
                    ot[0:100, :h],
                    ops[0:100, :h],
                    kv[0:100],
                    None,
                    mybir.AluOpType.add,
                )
                oc = c0 // 2
                nc.sync.dma_start(o_d[0:36, oc : oc + h], ot[0:36, :h])
                nc.sync.dma_start(o_d[36:72, oc : oc + h], ot[64:100, :h])
    nc.compile()
    return nc


def _coeffs(a: np.ndarray):
    """Host-side prep of the constant matrices from param a [96,3]."""
    a_eff = a.reshape(6, 16, 3).sum(0)  # [16,3]
    A = a_eff.T.reshape(3, 4, 4)  # A[c] with out_c = z^T A_c z
    As = 0.5 * (A + A.transpose(0, 2, 1))  # symmetrize
    Q = As[:, :3, :3]  # [3,3,3] quadratic part
    L = 2.0 * As[:, :3, 3]  # [3,3] linear coefs
    K = As[:, 3, 3].copy()  # [3] constants
    # guard tiny K (u_c = L_c / (2 K_c)); shift the constant via kv fold
    Ksafe = np.where(np.abs(K) < 1e-3, 1.0, K)
    U = L / (2.0 * Ksafe[:, None])  # [3,3] tailored directions

    # basis quadratic parts: M[s] (3x3 sym) for s=0..8
    E = np.eye(3, dtype=np.float64)
    dirs = [
        (E[0], E[0]), (E[1], E[1]), (E[2], E[2]),
        (E[0] + E[1], E[0] + E[1]),
        (E[0] + E[2], E[0] + E[2]),
        (E[1] + E[2], E[1] + E[2]),
    ]
    M = np.zeros((9, 3, 3))
    for s, (u, v) in enumerate(dirs):
        M[s] = np.outer(u, v)
    for c in range(3):
        M[6 + c] = np.outer(U[c], U[c])
    # solve for weights: Q[c] = sum_s w[c,s] M[s] with constraints:
    # w[c,6+c'] = Ksafe[c] if c'==c else 0  (the tailored square carries
    # the linear term with weight K so 2*w*u = L)
    Mflat = M.reshape(9, 9)[:, [0, 1, 2, 4, 5, 8, 1, 2, 5]]
    # use upper-tri representation: entries (00,11,22,01,02,12) with
    # off-diag doubled
    def sym6(S):
        return np.array(
            [S[0, 0], S[1, 1], S[2, 2], S[0, 1] + S[1, 0],
             S[0, 2] + S[2, 0], S[1, 2] + S[2, 1]]
        )

    B6 = np.stack([sym6(M[s]) for s in range(9)])  # [9,6]
    W = np.zeros((3, 9))
    for c in range(3):
        rhs = sym6(Q[c]) - Ksafe[c] * B6[6 + c]
        W[c, :6] = np.linalg.solve(B6[:6].T, rhs)
        W[c, 6 + c] = Ksafe[c]
    # constant leftover: out_c = sum_s W[c,s] q_s + kconst[c]
    # tailored square contributes Ksafe*1 at x=0... full check:
    # value at x=0: sum_s W[c,s]*(bias_s)^2 = W[c,6+c]*1 = Ksafe[c]
    kconst = K - Ksafe
    return U, W, kconst


def _host_tensors(a: np.ndarray):
    U, W, kconst = _coeffs(a.astype(np.float64))
    pv = np.zeros((36, 108), dtype=np.float32)
    bias = np.zeros((108, 1), dtype=np.float32)
    ab = np.zeros((108, 36), dtype=np.float32)
    kv = np.zeros((128, 1), dtype=np.float32)
    forms = [
        [(0, 1.0)], [(1, 1.0)], [(2, 1.0)],
        [(0, 1.0), (1, 1.0)], [(0, 1.0), (2, 1.0)], [(1, 1.0), (2, 1.0)],
    ]
    for g in range(G):
        for s in range(9):
            col = 9 * g + s
            if s < 6:
                for j, v in forms[s]:
                    pv[3 * g + j, col] = v
            else:
                c = s - 6
                for j in range(3):
                    pv[3 * g + j, col] = U[c, j]
                bias[col, 0] = 1.0
        for c in range(3):
            orow = 3 * g + c
            for s in range(9):
                ab[9 * g + s, orow] = W[c, s]
    for g in range(G):
        for c in range(3):
            kv[3 * g + c, 0] = kconst[c]
            kv[64 + 3 * g + c, 0] = kconst[c]
    return pv, ab, bias, kv


def kernel(x: np.ndarray, a: np.ndarray) -> np.ndarray:
    x = np.ascontiguousarray(x, dtype=np.float32)
    a = np.ascontiguousarray(a, dtype=np.float32)
    if "nc" not in _CACHE:
        _CACHE["nc"] = _build_nc()
    nc = _CACHE["nc"]

    pv, ab, bias, kv = _host_tensors(a)
    in_maps = []
    for ci in range(N_CORES):
        xs = x[ci * N_PER_CORE : (ci + 1) * N_PER_CORE]
        xp = np.zeros((NPAD, 3), dtype=np.float32)
        xp[:N_PER_CORE] = xs
        z = np.ascontiguousarray(
            xp.reshape(FTOT, G, 3).transpose(1, 2, 0).reshape(36, FTOT)
        )
        in_maps.append({"z": z, "pv": pv, "ab": ab, "bias": bias, "kv": kv})

    res = run_bass_kernel_spmd(nc, in_maps, list(range(N_CORES)))

    out = np.empty((N_POINTS, 3), dtype=np.float32)
    H = FT  # half-macro width
    for ci in range(N_CORES):
        o = res.results[ci]["o"]  # [72, FTOT//2]
        full = np.empty((NPAD, 3), dtype=np.float32)
        # column m of Z maps: rows[3g+c] of half h -> point 12*m_global+g
        # macro t covers Z cols [2*FT*t, 2*FT*t+w); half0 -> o rows 0:36 at
        # o-cols [FT*t ...], half1 -> o rows 36:72
        ov = o.reshape(2, G, 3, FTOT // 2)  # [half_rows, g, c, ocol]
        ncols_half = FTOT // 2
        # Build m_global for each (half, ocol): m = 2*FT*t + h*half_w + k
        # where ocol = FT*t + k, half_w = w//2. For full tiles half_w = FT.
        # Tail tile (w < 2*FT) also has half_w = w//2 = h_tail and its ocols
        # span [FT*t, FT*t + h_tail).
        mcols = np.empty((2, ncols_half), dtype=np.int64)
        for t in range(NTILES):
            c0 = 2 * FT * t
            w = min(2 * FT, FTOT - c0)
            h = w // 2
            oc = c0 // 2
            k = np.arange(h)
            mcols[0, oc : oc + h] = c0 + k
            mcols[1, oc : oc + h] = c0 + h + k
        for half in range(2):
            m = mcols[half]  # [ncols_half]
            pts = (G * m[:, None] + np.arange(G)[None, :]).ravel()  # [ncols*G]
            vals = ov[half].transpose(2, 0, 1).reshape(ncols_half * G, 3)
            full[pts] = vals
        out[ci * N_PER_CORE : (ci + 1) * N_PER_CORE] = full[:N_PER_CORE]
    return out



# revision 4
# speedup vs baseline: 1.5634x; 1.5634x over previous
"""Trainium2 Bass kernel for nn_Grid_fun: out = tile(feat(z), 6) @ a.

Math: z = [x, 1] (N,4); feat = (z otimes z).reshape(N,16); out = tile(feat,6) @ a
    = feat @ a_eff  where a_eff = a.reshape(6,16,3).sum(0)   [16,3]
    => out[n,c] = z[n]^T A_c z[n],  A_c = a_eff[:,c].reshape(4,4)

Device algorithm (per core, data-parallel over N, all-bf16 matmuls):
  Host stages x as Z[3g+j, m] = x[14 m + g, j]  (G=14 groups x 3 comps = 42
  partition rows, points along the free dim, bf16).
  mm1:  V[127,F] = pv^T @ Z      (9 forms per group + 1 shared zero row)
        forms: x0,x1,x2, x0+x1,x0+x2,x1+x2, x0,x1,x2(+bias), 0(+bias)
  ACT/DVE: R = Square(V + bias)  bf16   (bias=1 on the last-3-per-group rows
        and the shared row -> (x_j+1)^2 and the constant 1)
  mm2:  O[42,F] = ab^T @ R       per group: out_c = sum_s W[c,s] R_s + K'_c*1
        (universal closed-form weights; constants folded via the ones row)
  Three consecutive tiles' outputs pack into one PSUM tile [126, 512];
  one DVE copy drains them to bf16 SBUF; 3 large DMAs write DRAM.
"""

import sys

if "/opt/trn_rl_repo" not in sys.path:
    sys.path.insert(0, "/opt/trn_rl_repo")

from contextlib import ExitStack

import ml_dtypes
import numpy as np

import concourse.bass as bass
import concourse.mybir as mybir
import concourse.tile as tile
from concourse import bacc
from concourse.bass_utils import run_bass_kernel_spmd

N_CORES = 8
N_POINTS = 1_000_000
N_PER_CORE = N_POINTS // N_CORES  # 125000
G = 14  # points (groups) per column
ZR = 3 * G  # 42 partition rows of Z
VR = 9 * G + 1  # 127 = form rows + shared ones row
T = 512  # matmul free-dim tile
NT = 18  # tiles per core
FTOT = NT * T  # 9216 columns per core
NPAD = G * FTOT  # 129024 >= N_PER_CORE
SUP = 2  # tiles packed per PSUM output super-tile (PE bases 0 and 64)
NSUP = NT // SUP  # 9 super-tiles
ABW = 64  # ab stationary width; cols 42:64 are zero (pads rows 42:64)
OROW = 64 + ZR  # 106 rows in the packed output tile
NCH = 6  # input DMA chunks
CHUNK = FTOT // NCH  # 1536 columns per input chunk
OCH = 3 * T  # output DMA chunk width (3 super-tiles)
WS = 488  # columns squared on the Scalar engine (rest on DVE)

BF16 = ml_dtypes.bfloat16

_CACHE: dict = {}


def _build_nc():
    nc = bacc.Bacc("TRN2", target_bir_lowering=False)
    f32 = mybir.dt.float32
    bf16 = mybir.dt.bfloat16

    z_d = nc.dram_tensor("z", [ZR, FTOT], bf16, kind="ExternalInput")
    pv_d = nc.dram_tensor("pv", [ZR, VR], bf16, kind="ExternalInput")
    ab_d = nc.dram_tensor("ab", [VR, ABW], bf16, kind="ExternalInput")
    bias_d = nc.dram_tensor("bias", [VR, 1], f32, kind="ExternalInput")
    o_d = nc.dram_tensor("o", [OROW, NSUP * T], bf16, kind="ExternalOutput")

    sq = mybir.ActivationFunctionType.Square
    add = mybir.AluOpType.add
    mult = mybir.AluOpType.mult

    with tile.TileContext(nc) as tc:
        with ExitStack() as ctx:
            cpool = ctx.enter_context(tc.tile_pool(name="consts", bufs=1))
            rpool = ctx.enter_context(tc.tile_pool(name="rt", bufs=3))
            tpool = ctx.enter_context(tc.tile_pool(name="tb", bufs=3))
            vpool = ctx.enter_context(
                tc.tile_pool(name="vps", bufs=3, space="PSUM")
            )
            opool = ctx.enter_context(
                tc.tile_pool(name="ops", bufs=2, space="PSUM")
            )

            pv = cpool.tile([ZR, VR], bf16)
            ab = cpool.tile([VR, ABW], bf16)
            bias = cpool.tile([VR, 1], f32)
            o_sb = cpool.tile([OROW, NSUP * T], bf16)
            zc = [
                cpool.tile([ZR, CHUNK], bf16, name=f"zc{k}")
                for k in range(NCH)
            ]

            # const + input DMAs: gpsimd carries pv/ab (+ output later),
            # sync carries bias + the 6 input chunks.
            nc.gpsimd.dma_start(pv[:], pv_d[:, :])
            nc.sync.dma_start(zc[0][:], z_d[:, 0:CHUNK])
            nc.gpsimd.dma_start(ab[:], ab_d[:, :])
            nc.sync.dma_start(bias[:], bias_d[:, :])
            for k in range(1, NCH):
                nc.sync.dma_start(zc[k][:], z_d[:, k * CHUNK : (k + 1) * CHUNK])

            ops = None
            for t in range(NT):
                s, b = divmod(t, SUP)
                k, kb = divmod(t, 3)
                vps = vpool.tile([VR, T], f32)
                nc.tensor.matmul(
                    vps[:], pv[:], zc[k][:, kb * T : (kb + 1) * T],
                    start=True, stop=True,
                )

                rt = rpool.tile([VR, T], bf16)
                nc.scalar.activation(
                    rt[:, :WS], vps[:, :WS], sq, bias=bias[:], scale=1.0
                )
                tb = tpool.tile([VR, T - WS], bf16)
                nc.vector.tensor_scalar(tb[:], vps[:, WS:], bias[:], None, add)
                nc.vector.tensor_tensor(rt[:, WS:], tb[:], tb[:], mult)

                if b == 0:
                    ops = opool.tile([OROW, T], f32)
                    nc.tensor.matmul(
                        ops[0:ABW, :], ab[:], rt[:], start=True, stop=True
                    )
                else:
                    nc.tensor.matmul(
                        ops[ABW : ABW + ZR, :], ab[:, :ZR], rt[:],
                        start=True, stop=True,
                    )
                    nc.vector.tensor_scalar(
                        o_sb[:, s * T : (s + 1) * T], ops[:], 0.0, None, add
                    )
                if t % (NT // 3) == NT // 3 - 1:
                    j = t // (NT // 3)
                    nc.gpsimd.dma_start(
                        o_d[:, j * OCH : (j + 1) * OCH],
                        o_sb[:, j * OCH : (j + 1) * OCH],
                    )
    nc.compile()
    return nc


def _host_tensors(a: np.ndarray):
    """pv / ab / bias from param a [96,3] (exact closed form, fp64)."""
    a_eff = a.astype(np.float64).reshape(6, 16, 3).sum(0)  # [16,3]
    A = a_eff.T.reshape(3, 4, 4)
    As = 0.5 * (A + A.transpose(0, 2, 1))
    Q = As[:, :3, :3]  # [3,3,3] quadratic part
    L = 2.0 * As[:, :3, 3]  # [3,3] linear coefs
    K = As[:, 3, 3]  # [3] constants

    pairs = [(0, 1), (0, 2), (1, 2)]
    W = np.zeros((3, 9))
    for c in range(3):
        for p, (j, k) in enumerate(pairs):
            W[c, 3 + p] = Q[c, j, k]
        for j in range(3):
            W[c, 6 + j] = 0.5 * L[c, j]
            W[c, j] = (
                Q[c, j, j]
                - sum(Q[c, j, k] for k in range(3) if k != j)
                - 0.5 * L[c, j]
            )
    Wones = K - 0.5 * L.sum(axis=1)  # [3]

    pv = np.zeros((ZR, VR), dtype=np.float32)
    bias = np.zeros((VR, 1), dtype=np.float32)
    ab = np.zeros((VR, ABW), dtype=np.float32)
    for g in range(G):
        for j in range(3):
            pv[3 * g + j, 9 * g + j] = 1.0  # x_j
            pv[3 * g + j, 9 * g + 6 + j] = 1.0  # x_j (+1 bias)
            bias[9 * g + 6 + j, 0] = 1.0
        for p, (j, k) in enumerate(pairs):
            pv[3 * g + j, 9 * g + 3 + p] = 1.0  # x_j + x_k
            pv[3 * g + k, 9 * g + 3 + p] = 1.0
        for c in range(3):
            for ss in range(9):
                ab[9 * g + ss, 3 * g + c] = W[c, ss]
            ab[VR - 1, 3 * g + c] = Wones[c]
    bias[VR - 1, 0] = 1.0  # shared ones row
    return pv.astype(BF16), ab.astype(BF16), bias


def _stage_x(x: np.ndarray, ci: int) -> np.ndarray:
    xs = x[ci * N_PER_CORE : (ci + 1) * N_PER_CORE]
    xp = np.zeros((NPAD, 3), dtype=np.float32)
    xp[:N_PER_CORE] = xs
    z = xp.reshape(FTOT, G, 3).transpose(1, 2, 0).reshape(ZR, FTOT)
    return np.ascontiguousarray(z).astype(BF16)


def _decode_o(o: np.ndarray) -> np.ndarray:
    """o [106, 4608] bf16 -> [N_PER_CORE, 3] fp32."""
    of = o.astype(np.float32)
    tmp = np.stack([of[0:ZR], of[ABW : ABW + ZR]])  # [b, 42, 4608]
    o5 = tmp.reshape(SUP, G, 3, NSUP, T)  # [b,g,c,s,w]
    full = o5.transpose(3, 0, 4, 1, 2).reshape(NPAD, 3)  # m = 1024s+512b+w
    return full[:N_PER_CORE]


def kernel(x: np.ndarray, a: np.ndarray) -> np.ndarray:
    x = np.ascontiguousarray(x, dtype=np.float32)
    a = np.ascontiguousarray(a, dtype=np.float32)
    if "nc" not in _CACHE:
        _CACHE["nc"] = _build_nc()
    nc = _CACHE["nc"]

    pv, ab, bias = _host_tensors(a)
    in_maps = []
    for ci in range(N_CORES):
        in_maps.append(
            {"z": _stage_x(x, ci), "pv": pv, "ab": ab, "bias": bias}
        )

    res = run_bass_kernel_spmd(nc, in_maps, list(range(N_CORES)))

    out = np.empty((N_POINTS, 3), dtype=np.float32)
    for ci in range(N_CORES):
        out[ci * N_PER_CORE : (ci + 1) * N_PER_CORE] = _decode_o(
            res.results[ci]["o"]
        )
    return out
